# revision 12
# baseline (speedup 1.0000x reference)
"""KAN layer (Chebyshev deg-8) Trainium2 kernel, 8-core data-parallel.

Math: out[b] = sum_n hw[n] * (X @ C.T)[b,n] = X[b,:] @ (C.T @ hw)
            = sum_d sum_k W[d,k] * T_k(tanh(x[b,d])),  W[d,k]=(C.T@hw)[d*9+k]

Device evaluates 8 polynomial streams per element (degrees 1..8 in
u = tanh(x)) and contracts them against per-dim weights on the PE.
v3 schedule: c-fused tiles ([128, 4096] free = [d0-127 | d128-255] of one
2048-col batch chunk) halve per-op overhead; hybrid ACT/DVE DAG keeps both
engines dense from t0 (q on DVE so DVE starts right after the first tanh;
squares s4/s44/w6 on ACT).  The second batch chunk swaps its degree-6
stream from q*s4 (DVE product) to (uq)^2 (ACT Square) to rebalance engine
load — weights are per-chunk so the basis can differ per chunk.
  chunk0: ACT tanh,s4=(2q-1)^2,s44a | DVE q,uq,us4,qs4,uqs4,s44b
  chunk1: ACT tanh,s4,w6=(uq)^2,s44 | DVE q,uq,us4,uqs4
The d-contraction runs on the PE as per-stream matvecs with 4 batch
sub-blocks concurrent via column tiling (PSUM rows 0/32/64/96 of one bank).
Host: transposes x to [D, B] fp16, folds hweights into coeffs, solves the
per-chunk stream->Chebyshev transforms with fp16-rounding compensation, and
adds the T0 constant plus output-row gather on the way out.
"""
import sys
import numpy as np

sys.path.insert(0, "/opt/trn_rl_repo")

import orjson
from contextlib import ExitStack

import concourse.bass as bass
from concourse import mybir
from concourse.tile import TileContext
from concourse.bass_utils import run_bass_kernel_spmd

F32 = mybir.dt.float32
F16 = mybir.dt.float16
AF = mybir.ActivationFunctionType
OP = mybir.AluOpType

B, D, DEG1 = 32768, 256, 9
NCORES = 8
BC = B // NCORES          # 4096 batch per core
SBW = 2048                # batch cols per super-block (chunk)
NSB = BC // SBW           # 2 chunks per core
FUS = 2 * SBW             # fused free dim: [0,SBW)=dims 0-127, [SBW,2SBW)=dims 128-255
NGRP = 4                  # PE column groups (batch sub-blocks in flight)
SUB = SBW // NGRP         # 512 cols per sub-block == one PSUM bank row

SLOTS = ["u", "q", "uq", "s4", "us4", "d6", "uqs4", "s44"]

# ---- walrus workaround: split >1 sem-waits onto Drain carriers -------------
_MAXW = 1

def _split_waits(bir_json: bytes) -> bytes:
    d = orjson.loads(bir_json)
    for fn in d.get("functions", []):
        for bb in fn.get("blocks", []):
            out = []
            for ins in bb.get("instructions", []):
                si = ins.get("sync_info") or {}
                waits = si.get("on_wait") or []
                if len(waits) > _MAXW:
                    extra, keep = waits[:-_MAXW], waits[-_MAXW:]
                    for i in range(0, len(extra), _MAXW):
                        out.append({
                            "debug": ins.get("debug", 0),
                            "engine": ins["engine"], "ins": [], "outs": [],
                            "name": f"{ins['name']}_ws{i}", "opcode": "Drain",
                            "sync_info": {"on_update": [],
                                          "on_wait": extra[i:i + _MAXW]},
                        })
                    si["on_wait"] = keep
                out.append(ins)
            bb["instructions"] = out
    return orjson.dumps(d)

def _install_patch():
    import concourse.bass_utils as bu
    if getattr(bu, "_ws_patched", False):
        return
    orig = bu.compile_bir_kernel
    def patched(bir_json, tmpdir, neff_name="file.neff"):
        return orig(_split_waits(bir_json), tmpdir, neff_name)
    bu.compile_bir_kernel = patched
    bu._ws_patched = True
    try:
        import concourse.bass2jax as b2j
        if getattr(b2j, "compile_bir_kernel", None) is orig:
            b2j.compile_bir_kernel = patched
    except Exception:
        pass

# ---- basis transform (host) ------------------------------------------------
def _stream_polys(sb):
    """Power-basis coefficients (in u) of each stream, index by degree 1..8."""
    P = np.polynomial.polynomial
    u = [0.0, 1.0]
    q = P.polymul(u, u)
    uq = P.polymul(u, q)
    t2 = P.polyadd(P.polymul([2.0], q), [-1.0])     # 2u^2 - 1
    s4 = P.polymul(t2, t2)
    us4 = P.polymul(u, s4)
    d6 = P.polymul(q, s4) if sb == 0 else P.polymul(uq, uq)
    uqs4 = P.polymul(uq, s4)
    s44 = P.polymul(s4, s4)
    return {1: u, 2: q, 3: uq, 4: s4, 5: us4, 6: d6, 7: uqs4, 8: s44}

def _basis_matrix(sb):
    from numpy.polynomial import chebyshev as C
    A = np.zeros((9, 9))
    A[0, 0] = 1.0
    for t, poly in _stream_polys(sb).items():
        c = C.poly2cheb(poly)
        A[: len(c), t] = c
    return A

# ---- device kernel ---------------------------------------------------------
def _build():
    nc = bass.Bass(enable_partition_id=False)
    xt = nc.declare_dram_parameter("xt", [D, BC], F16, isOutput=False)
    wv = nc.declare_dram_parameter("wv", [128, 32], F16, isOutput=False)
    y = nc.declare_dram_parameter("y", [NGRP, NSB * SUB], F32, isOutput=True)

    with TileContext(nc) as tc, ExitStack() as ctx:
        fp = ctx.enter_context(tc.tile_pool(name="feat", bufs=1))
        pp = ctx.enter_context(tc.tile_pool(name="ps", bufs=2, space="PSUM"))

        Q1 = 512
        # input DMAs in consumption order; first transfer small so its
        # completion semaphore fires early
        xf = {}
        for sb in range(NSB):
            xf[sb] = fp.tile([128, FUS], F16, tag=f"x{sb}", name=f"xf{sb}")
        # first chunk's DMA rides the scalar queue: the ACT table load that
        # follows it hides the DMA latency, and it avoids sharing the sync
        # queue's round-robin bandwidth with the bulk transfers
        nc.scalar.dma_start(out=xf[0][:, 0:Q1], in_=xt[0:128, 0:Q1])
        eng = nc.sync
        eng.dma_start(out=xf[0][:, Q1:SBW], in_=xt[0:128, Q1:SBW])
        eng.dma_start(out=xf[0][:, SBW:FUS], in_=xt[128:256, 0:SBW])
        eng.dma_start(out=xf[1][:, 0:SBW], in_=xt[0:128, SBW:BC])
        eng.dma_start(out=xf[1][:, SBW:FUS], in_=xt[128:256, SBW:BC])

        wb = fp.tile([128, 32], F16, tag="wb")
        nc.gpsimd.dma_start(out=wb[:], in_=wv[:])
        bM1 = fp.tile([128, 1], F32, tag="bM1")
        nc.gpsimd.memset(bM1[:], -1.0)

        res = fp.tile([128, NSB * SUB], F32, tag="res")
        S = {sb: {} for sb in range(NSB)}
        for sb in range(NSB):
            for nm in SLOTS:
                S[sb][nm] = fp.tile([128, FUS], F16, tag=f"{nm}{sb}",
                                    name=f"{nm}_{sb}")

        u0, q0 = S[0]["u"], S[0]["q"]
        u1, q1 = S[1]["u"], S[1]["q"]

        # ---- stream builds, emitted in topological (execution) order so
        # Tile's program-order dependency tracking sees writer-before-reader;
        # the induced per-engine queue orders keep both engines dense ----
        nc.scalar.activation(u0[:, 0:Q1], xf[0][:, 0:Q1], AF.Tanh)
        nc.vector.tensor_mul(q0[:, 0:Q1], u0[:, 0:Q1], u0[:, 0:Q1])
        nc.scalar.activation(u0[:, Q1:SBW], xf[0][:, Q1:SBW], AF.Tanh)
        nc.vector.tensor_mul(q0[:, Q1:SBW], u0[:, Q1:SBW], u0[:, Q1:SBW])
        nc.scalar.activation(u0[:, SBW:FUS], xf[0][:, SBW:FUS], AF.Tanh)
        nc.vector.tensor_mul(q0[:, SBW:FUS], u0[:, SBW:FUS], u0[:, SBW:FUS])
        nc.scalar.activation(u1[:], xf[1][:], AF.Tanh)
        nc.vector.tensor_mul(S[0]["uq"][:], u0[:], q0[:])
        nc.vector.tensor_mul(q1[:], u1[:], u1[:])
        nc.scalar.activation(S[0]["s4"][:], q0[:], AF.Square,
                             bias=bM1[:], scale=2.0)
        nc.vector.tensor_mul(S[1]["uq"][:], u1[:], q1[:])
        nc.scalar.activation(S[0]["s44"][:, 0:SBW], S[0]["s4"][:, 0:SBW],
                             AF.Square)
        nc.scalar.activation(S[1]["s4"][:], q1[:], AF.Square,
                             bias=bM1[:], scale=2.0)
        nc.vector.tensor_mul(S[0]["us4"][:], u0[:], S[0]["s4"][:])
        nc.vector.tensor_mul(S[0]["d6"][:], q0[:], S[0]["s4"][:])
        nc.scalar.activation(S[1]["d6"][:], S[1]["uq"][:], AF.Square)
        nc.vector.tensor_mul(S[0]["uqs4"][:], S[0]["uq"][:], S[0]["s4"][:])
        nc.scalar.activation(S[1]["s44"][:], S[1]["s4"][:], AF.Square)
        nc.vector.tensor_mul(S[0]["s44"][:, SBW:FUS], S[0]["s4"][:, SBW:FUS],
                             S[0]["s4"][:, SBW:FUS])
        nc.vector.tensor_mul(S[1]["us4"][:], u1[:], S[1]["s4"][:])
        nc.vector.tensor_mul(S[1]["uqs4"][:, 0:SBW], S[1]["uq"][:, 0:SBW],
                             S[1]["s4"][:, 0:SBW])
        H = SBW // 2
        nc.vector.tensor_mul(S[1]["uqs4"][:, SBW:SBW + H],
                             S[1]["uq"][:, SBW:SBW + H],
                             S[1]["s4"][:, SBW:SBW + H])
        nc.vector.tensor_mul(S[1]["uqs4"][:, SBW + H:FUS],
                             S[1]["uq"][:, SBW + H:FUS],
                             S[1]["s4"][:, SBW + H:FUS])

        # ---- PE contraction: 16 (slot, c) rounds per chunk, interleaved
        # across chunks by stream availability; 4 col-group matvecs/round ----
        ps = {sb: pp.tile([128, SUB], F32, name=f"ps{sb}") for sb in range(NSB)}
        rcount = {0: 0, 1: 0}

        def rounds(sb, slot, cs=(0, 1), gs=None, stop=False):
            for c in cs:
                first = rcount[sb] == 0
                rcount[sb] += 1
                stream = S[sb][slot]
                for g in (gs if gs is not None else range(NGRP)):
                    nc.tensor.matmul(
                        ps[sb][32 * g:32 * g + 1, :],
                        wb[:, sb * 16 + c * 8 + SLOTS.index(slot):
                           sb * 16 + c * 8 + SLOTS.index(slot) + 1],
                        stream[:, c * SBW + g * SUB:c * SBW + (g + 1) * SUB],
                        start=first, stop=stop,
                        skip_group_check=True,
                        tile_position=(0, 32 * g))

        for slot in ["u", "q", "uq", "s4"]:
            rounds(0, slot)
        rounds(0, "s44", cs=(0,))
        for slot in ["us4", "d6"]:
            rounds(0, slot)
        for slot in ["u", "q", "uq", "s4"]:
            rounds(1, slot)
        rounds(0, "uqs4")
        rounds(0, "s44", cs=(1,), stop=True)   # last chunk-0 round
        for slot in ["d6", "us4", "s44"]:
            rounds(1, slot)
        rounds(1, "uqs4", cs=(0,))
        # final round split by group pair so it flushes with the two
        # half-width DVE ops finishing uqs4 chunk-1
        rounds(1, "uqs4", cs=(1,), gs=(0, 1), stop=True)
        rcount[1] -= 1
        rounds(1, "uqs4", cs=(1,), gs=(2, 3), stop=True)
        # (copies emitted below read disjoint PSUM row ranges, so the
        # groups-0/1 copy overlaps the groups-2/3 matmul flush)

        # chunk-0 result copies out whole; chunk-1 is split by PSUM row
        # halves so the first half (groups 0/1) copies and DMAs while the
        # final MM pair for groups 2/3 is still flushing
        nc.scalar.activation(res[:, 0:SUB], ps[0][:], AF.Identity)
        nc.sync.dma_start(out=y[:, 0:SUB], in_=res[0:128:32, 0:SUB])
        nc.scalar.activation(res[0:64, SUB:2 * SUB], ps[1][0:64, :],
                             AF.Identity)
        nc.sync.dma_start(out=y[0:2, SUB:2 * SUB],
                          in_=res[0:64:32, SUB:2 * SUB])
        nc.scalar.activation(res[64:128, SUB:2 * SUB], ps[1][64:128, :],
                             AF.Identity)
        nc.sync.dma_start(out=y[2:4, SUB:2 * SUB],
                          in_=res[64:128:32, SUB:2 * SUB])
    return nc

# ---- public entry ----------------------------------------------------------
def kernel(x, coeffs, hweights, _trace=False):
    _install_patch()
    x = np.asarray(x, dtype=np.float32)
    w = (coeffs.astype(np.float64).T @ hweights.astype(np.float64))  # [2304]
    W = w.reshape(D, DEG1)                                           # [d, k]
    # quantization-compensated solve per chunk basis: peel leading Chebyshev
    # components in decreasing degree; each stream's fp16 weight rounding is
    # re-absorbed by lower-degree streams, leftover T0 becomes a constant.
    wv = np.zeros((128, 32), dtype=np.float16)
    c0s = []
    for sb in range(NSB):
        A = _basis_matrix(sb)
        Wc = W.astype(np.float64).copy()
        lam = np.zeros((D, DEG1))
        for t in range(DEG1 - 1, 0, -1):
            lt = Wc[:, t] / A[t, t]
            ltq = lt.astype(np.float16).astype(np.float64)
            Wc -= ltq[:, None] * A[:, t][None, :]
            lam[:, t] = ltq
        c0s.append(float(Wc[:, 0].sum()))
        for c in range(2):
            for sidx in range(8):
                wv[:, sb * 16 + c * 8 + sidx] = lam[c * 128:(c + 1) * 128,
                                                    sidx + 1]

    nc = _build()
    xT = np.ascontiguousarray(x.T.astype(np.float16))                # [D, B]
    in_maps = [{"xt": np.ascontiguousarray(xT[:, i * BC:(i + 1) * BC]),
                "wv": wv} for i in range(NCORES)]
    res = run_bass_kernel_spmd(nc, in_maps, core_ids=list(range(NCORES)),
                               trace=_trace)
    # y[g, sb*SUB + i] holds batch col sb*SBW + g*SUB + i of this core;
    # the T0 constant (per-chunk) is added here (global, so host-side is free)
    parts = []
    for i in range(NCORES):
        yc = res.results[i]["y"].astype(np.float64)        # [NGRP, NSB*SUB]
        yc = yc.reshape(NGRP, NSB, SUB)
        for sb in range(NSB):
            yc[:, sb, :] += c0s[sb]
        parts.append(yc.transpose(1, 0, 2).reshape(BC))
    out = np.concatenate(parts)
    if _trace:
        kernel._last = res
    return out.astype(np.float32)


# revision 13
# speedup vs baseline: 1.0439x; 1.0439x over previous
"""KAN layer (Chebyshev deg-8) Trainium2 kernel, 8-core data-parallel.

Math: out[b] = sum_n hw[n] * (X @ C.T)[b,n] = X[b,:] @ (C.T @ hw)
            = sum_d sum_k W[d,k] * T_k(tanh(x[b,d])),  W[d,k]=(C.T@hw)[d*9+k]

Device evaluates 8 polynomial streams per element (degrees 1..8 in
u = tanh(x)) and contracts them against per-dim weights on the PE.
v3 schedule: c-fused tiles ([128, 4096] free = [d0-127 | d128-255] of one
2048-col batch chunk) halve per-op overhead; hybrid ACT/DVE DAG keeps both
engines dense from t0 (q on DVE so DVE starts right after the first tanh;
squares s4/s44/w6 on ACT).  The second batch chunk swaps its degree-6
stream from q*s4 (DVE product) to (uq)^2 (ACT Square) to rebalance engine
load — weights are per-chunk so the basis can differ per chunk.
  chunk0: ACT tanh,s4=(2q-1)^2,s44a | DVE q,uq,us4,qs4,uqs4,s44b
  chunk1: ACT tanh,s4,w6=(uq)^2,s44 | DVE q,uq,us4,uqs4
The d-contraction runs on the PE as per-stream matvecs with 4 batch
sub-blocks concurrent via column tiling (PSUM rows 0/32/64/96 of one bank).
Host: transposes x to [D, B] fp16, folds hweights into coeffs, solves the
per-chunk stream->Chebyshev transforms with fp16-rounding compensation, and
adds the T0 constant plus output-row gather on the way out.
"""
import sys
import numpy as np

sys.path.insert(0, "/opt/trn_rl_repo")

import orjson
from contextlib import ExitStack

import concourse.bass as bass
from concourse import mybir
from concourse.tile import TileContext
from concourse.bass_utils import run_bass_kernel_spmd

F32 = mybir.dt.float32
F16 = mybir.dt.float16
AF = mybir.ActivationFunctionType
OP = mybir.AluOpType

B, D, DEG1 = 32768, 256, 9
NCORES = 8
BC = B // NCORES          # 4096 batch per core
SBW = 2048                # batch cols per super-block (chunk)
NSB = BC // SBW           # 2 chunks per core
FUS = 2 * SBW             # fused free dim: [0,SBW)=dims 0-127, [SBW,2SBW)=dims 128-255
NGRP = 4                  # PE column groups (batch sub-blocks in flight)
SUB = SBW // NGRP         # 512 cols per sub-block == one PSUM bank row

SLOTS = ["u", "q", "uq", "s4", "us4", "d6", "uqs4", "s44"]

# ---- walrus workaround: split >1 sem-waits onto Drain carriers -------------
_MAXW = 1

def _split_waits(bir_json: bytes) -> bytes:
    d = orjson.loads(bir_json)
    for fn in d.get("functions", []):
        for bb in fn.get("blocks", []):
            out = []
            for ins in bb.get("instructions", []):
                si = ins.get("sync_info") or {}
                waits = si.get("on_wait") or []
                if len(waits) > _MAXW:
                    extra, keep = waits[:-_MAXW], waits[-_MAXW:]
                    for i in range(0, len(extra), _MAXW):
                        out.append({
                            "debug": ins.get("debug", 0),
                            "engine": ins["engine"], "ins": [], "outs": [],
                            "name": f"{ins['name']}_ws{i}", "opcode": "Drain",
                            "sync_info": {"on_update": [],
                                          "on_wait": extra[i:i + _MAXW]},
                        })
                    si["on_wait"] = keep
                out.append(ins)
            bb["instructions"] = out
    return orjson.dumps(d)

def _install_patch():
    import concourse.bass_utils as bu
    if getattr(bu, "_ws_patched", False):
        return
    orig = bu.compile_bir_kernel
    def patched(bir_json, tmpdir, neff_name="file.neff"):
        return orig(_split_waits(bir_json), tmpdir, neff_name)
    bu.compile_bir_kernel = patched
    bu._ws_patched = True
    try:
        import concourse.bass2jax as b2j
        if getattr(b2j, "compile_bir_kernel", None) is orig:
            b2j.compile_bir_kernel = patched
    except Exception:
        pass

# ---- basis transform (host) ------------------------------------------------
def _stream_polys(sb):
    """Power-basis coefficients (in u) of each stream, index by degree 1..8."""
    P = np.polynomial.polynomial
    u = [0.0, 1.0]
    q = P.polymul(u, u)
    uq = P.polymul(u, q)
    t2 = P.polyadd(P.polymul([2.0], q), [-1.0])     # 2u^2 - 1
    s4 = P.polymul(t2, t2)
    us4 = P.polymul(u, s4)
    d6 = P.polymul(q, s4) if sb == 0 else P.polymul(uq, uq)
    uqs4 = P.polymul(uq, s4)
    s44 = P.polymul(s4, s4)
    return {1: u, 2: q, 3: uq, 4: s4, 5: us4, 6: d6, 7: uqs4, 8: s44}

def _basis_matrix(sb):
    from numpy.polynomial import chebyshev as C
    A = np.zeros((9, 9))
    A[0, 0] = 1.0
    for t, poly in _stream_polys(sb).items():
        c = C.poly2cheb(poly)
        A[: len(c), t] = c
    return A

# ---- device kernel ---------------------------------------------------------
def _build():
    nc = bass.Bass(enable_partition_id=False)
    xt = nc.declare_dram_parameter("xt", [D, BC], F16, isOutput=False)
    wv = nc.declare_dram_parameter("wv", [128, 32], F16, isOutput=False)
    y = nc.declare_dram_parameter("y", [NGRP, NSB * SUB], F32, isOutput=True)

    with TileContext(nc) as tc, ExitStack() as ctx:
        fp = ctx.enter_context(tc.tile_pool(name="feat", bufs=1))
        pp = ctx.enter_context(tc.tile_pool(name="ps", bufs=2, space="PSUM"))

        Q1 = 512
        # input DMAs in consumption order; first transfer small so its
        # completion semaphore fires early
        xf = {}
        for sb in range(NSB):
            xf[sb] = fp.tile([128, FUS], F16, tag=f"x{sb}", name=f"xf{sb}")
        # first chunk's DMA rides the scalar queue: the ACT table load that
        # follows it hides the DMA latency, and it avoids sharing the sync
        # queue's round-robin bandwidth with the bulk transfers
        nc.scalar.dma_start(out=xf[0][:, 0:Q1], in_=xt[0:128, 0:Q1])
        eng = nc.sync
        H = SBW // 2
        eng.dma_start(out=xf[0][:, Q1:SBW], in_=xt[0:128, Q1:SBW])
        eng.dma_start(out=xf[0][:, SBW:SBW + H], in_=xt[128:256, 0:H])
        eng.dma_start(out=xf[0][:, SBW + H:FUS], in_=xt[128:256, H:SBW])
        eng.dma_start(out=xf[1][:, 0:SBW], in_=xt[0:128, SBW:BC])
        eng.dma_start(out=xf[1][:, SBW:FUS], in_=xt[128:256, SBW:BC])

        wb = fp.tile([128, 32], F16, tag="wb")
        nc.gpsimd.dma_start(out=wb[:], in_=wv[:])
        bM1 = fp.tile([128, 1], F32, tag="bM1")
        nc.gpsimd.memset(bM1[:], -1.0)

        res = fp.tile([128, NSB * SUB], F32, tag="res")
        S = {sb: {} for sb in range(NSB)}
        for sb in range(NSB):
            for nm in SLOTS:
                S[sb][nm] = fp.tile([128, FUS], F16, tag=f"{nm}{sb}",
                                    name=f"{nm}_{sb}")

        u0, q0 = S[0]["u"], S[0]["q"]
        u1, q1 = S[1]["u"], S[1]["q"]

        # ---- stream builds, emitted in topological (execution) order so
        # Tile's program-order dependency tracking sees writer-before-reader;
        # the induced per-engine queue orders keep both engines dense ----
        nc.scalar.activation(u0[:, 0:Q1], xf[0][:, 0:Q1], AF.Tanh)
        nc.vector.tensor_mul(q0[:, 0:Q1], u0[:, 0:Q1], u0[:, 0:Q1])
        nc.scalar.activation(u0[:, Q1:SBW], xf[0][:, Q1:SBW], AF.Tanh)
        nc.vector.tensor_mul(q0[:, Q1:SBW], u0[:, Q1:SBW], u0[:, Q1:SBW])
        nc.scalar.activation(u0[:, SBW:SBW + H], xf[0][:, SBW:SBW + H],
                             AF.Tanh)
        nc.vector.tensor_mul(q0[:, SBW:SBW + H], u0[:, SBW:SBW + H],
                             u0[:, SBW:SBW + H])
        nc.scalar.activation(u0[:, SBW + H:FUS], xf[0][:, SBW + H:FUS],
                             AF.Tanh)
        nc.vector.tensor_mul(q0[:, SBW + H:FUS], u0[:, SBW + H:FUS],
                             u0[:, SBW + H:FUS])
        nc.scalar.activation(u1[:, 0:SBW], xf[1][:, 0:SBW], AF.Tanh)
        nc.vector.tensor_mul(S[0]["uq"][:], u0[:], q0[:])
        nc.vector.tensor_mul(q1[:, 0:SBW], u1[:, 0:SBW], u1[:, 0:SBW])
        nc.scalar.activation(u1[:, SBW:FUS], xf[1][:, SBW:FUS], AF.Tanh)
        nc.vector.tensor_mul(q1[:, SBW:FUS], u1[:, SBW:FUS], u1[:, SBW:FUS])
        nc.scalar.activation(S[0]["s4"][:], q0[:], AF.Square,
                             bias=bM1[:], scale=2.0)
        nc.vector.tensor_mul(S[1]["uq"][:], u1[:], q1[:])
        nc.scalar.activation(S[0]["s44"][:, 0:SBW], S[0]["s4"][:, 0:SBW],
                             AF.Square)
        nc.scalar.activation(S[1]["s4"][:], q1[:], AF.Square,
                             bias=bM1[:], scale=2.0)
        nc.vector.tensor_mul(S[0]["us4"][:], u0[:], S[0]["s4"][:])
        nc.vector.tensor_mul(S[0]["d6"][:], q0[:], S[0]["s4"][:])
        nc.scalar.activation(S[1]["d6"][:], S[1]["uq"][:], AF.Square)
        nc.vector.tensor_mul(S[0]["uqs4"][:], S[0]["uq"][:], S[0]["s4"][:])
        nc.scalar.activation(S[1]["s44"][:], S[1]["s4"][:], AF.Square)
        nc.vector.tensor_mul(S[0]["s44"][:, SBW:FUS], S[0]["s4"][:, SBW:FUS],
                             S[0]["s4"][:, SBW:FUS])
        nc.vector.tensor_mul(S[1]["us4"][:], u1[:], S[1]["s4"][:])
        nc.vector.tensor_mul(S[1]["uqs4"][:, 0:SBW], S[1]["uq"][:, 0:SBW],
                             S[1]["s4"][:, 0:SBW])
        nc.vector.tensor_mul(S[1]["uqs4"][:, SBW:SBW + H],
                             S[1]["uq"][:, SBW:SBW + H],
                             S[1]["s4"][:, SBW:SBW + H])
        nc.vector.tensor_mul(S[1]["uqs4"][:, SBW + H:FUS],
                             S[1]["uq"][:, SBW + H:FUS],
                             S[1]["s4"][:, SBW + H:FUS])

        # ---- PE contraction: 16 (slot, c) rounds per chunk, interleaved
        # across chunks by stream availability; 4 col-group matvecs/round ----
        ps = {sb: pp.tile([128, SUB], F32, name=f"ps{sb}") for sb in range(NSB)}
        rcount = {0: 0, 1: 0}

        def rounds(sb, slot, cs=(0, 1), gs=None, stop=False):
            for c in cs:
                first = rcount[sb] == 0
                rcount[sb] += 1
                stream = S[sb][slot]
                for g in (gs if gs is not None else range(NGRP)):
                    nc.tensor.matmul(
                        ps[sb][32 * g:32 * g + 1, :],
                        wb[:, sb * 16 + c * 8 + SLOTS.index(slot):
                           sb * 16 + c * 8 + SLOTS.index(slot) + 1],
                        stream[:, c * SBW + g * SUB:c * SBW + (g + 1) * SUB],
                        start=first, stop=stop,
                        skip_group_check=True,
                        tile_position=(0, 32 * g))

        for slot in ["u", "q", "uq", "s4"]:
            rounds(0, slot)
        rounds(0, "s44", cs=(0,))
        for slot in ["us4", "d6"]:
            rounds(0, slot)
        for slot in ["u", "q", "uq", "s4"]:
            rounds(1, slot)
        rounds(0, "uqs4")
        rounds(0, "s44", cs=(1,), stop=True)   # last chunk-0 round
        for slot in ["d6", "us4", "s44"]:
            rounds(1, slot)
        rounds(1, "uqs4", cs=(0,))
        # final round split by group pair so it flushes with the two
        # half-width DVE ops finishing uqs4 chunk-1
        rounds(1, "uqs4", cs=(1,), gs=(0, 1), stop=True)
        rcount[1] -= 1
        rounds(1, "uqs4", cs=(1,), gs=(2, 3), stop=True)
        # (copies emitted below read disjoint PSUM row ranges, so the
        # groups-0/1 copy overlaps the groups-2/3 matmul flush)

        for sb in range(NSB):
            dst = res[:, sb * SUB:(sb + 1) * SUB]
            nc.scalar.activation(dst, ps[sb][:], AF.Identity)
            nc.sync.dma_start(out=y[:, sb * SUB:(sb + 1) * SUB],
                              in_=res[0:128:32, sb * SUB:(sb + 1) * SUB])
    return nc

# ---- public entry ----------------------------------------------------------
def kernel(x, coeffs, hweights, _trace=False):
    _install_patch()
    x = np.asarray(x, dtype=np.float32)
    w = (coeffs.astype(np.float64).T @ hweights.astype(np.float64))  # [2304]
    W = w.reshape(D, DEG1)                                           # [d, k]
    # quantization-compensated solve per chunk basis: peel leading Chebyshev
    # components in decreasing degree; each stream's fp16 weight rounding is
    # re-absorbed by lower-degree streams, leftover T0 becomes a constant.
    wv = np.zeros((128, 32), dtype=np.float16)
    c0s = []
    for sb in range(NSB):
        A = _basis_matrix(sb)
        Wc = W.astype(np.float64).copy()
        lam = np.zeros((D, DEG1))
        for t in range(DEG1 - 1, 0, -1):
            lt = Wc[:, t] / A[t, t]
            ltq = lt.astype(np.float16).astype(np.float64)
            Wc -= ltq[:, None] * A[:, t][None, :]
            lam[:, t] = ltq
        c0s.append(float(Wc[:, 0].sum()))
        for c in range(2):
            for sidx in range(8):
                wv[:, sb * 16 + c * 8 + sidx] = lam[c * 128:(c + 1) * 128,
                                                    sidx + 1]

    nc = _build()
    xT = np.ascontiguousarray(x.T.astype(np.float16))                # [D, B]
    in_maps = [{"xt": np.ascontiguousarray(xT[:, i * BC:(i + 1) * BC]),
                "wv": wv} for i in range(NCORES)]
    res = run_bass_kernel_spmd(nc, in_maps, core_ids=list(range(NCORES)),
                               trace=_trace)
    # y[g, sb*SUB + i] holds batch col sb*SBW + g*SUB + i of this core;
    # the T0 constant (per-chunk) is added here (global, so host-side is free)
    parts = []
    for i in range(NCORES):
        yc = res.results[i]["y"].astype(np.float64)        # [NGRP, NSB*SUB]
        yc = yc.reshape(NGRP, NSB, SUB)
        for sb in range(NSB):
            yc[:, sb, :] += c0s[sb]
        parts.append(yc.transpose(1, 0, 2).reshape(BC))
    out = np.concatenate(parts)
    if _trace:
        kernel._last = res
    return out.astype(np.float32)
